# revision 10
# baseline (speedup 1.0000x reference)
"""Fused single-head attention (QKV proj + softmax*scale + AV) on 8 trn2 cores.

Reference computation (fp32):
    qkv = x @ W.T            x:[4,4096,768]  W:[192,768]
    q,k,v = split(qkv, 64)
    A = q @ k.T              (no pre-softmax scale)
    out = softmax(A) / 8 @ v

Sharding: core c handles batch b=c//2, query half qh=c%2 (2048 queries),
full 4096 keys of that batch. SPMD-uniform program: the host rolls the
key/value columns of x^T by qh*2048 so every core's own queries are
always columns 0:2048 (softmax is permutation-invariant over keys).

v2 design notes (vs v1 baseline at 125us):
  - Dual-row projection: D=768 split into 12 chunks of 64; even chunks
    live on SBUF partitions 0:64, odd on 64:128. The per-chunk proj
    matmuls contract only 64 partitions, so even/odd chunk matmuls run
    CONCURRENTLY in the two PE row-halves (tile_position row groups),
    halving projection wall time. The pair of PSUM partials is summed
    by the DVE add that replaces v1's PSUM->SBUF copy (same DVE cost).
  - Q duplication (scores rhs needs Q^T on both row halves) is done by
    duplicating the Q columns of W (M=128 lhsT) instead of a second
    DVE copy.
  - All input DMAs pre-issued at program start on the sync HWDGE queue
    in consumption order (wt, sb0 in 3 chunks, sb1..sb7 whole), so the
    queue streams back-to-back instead of being paced by emission.
  - ~3.5us of dummy identity matmuls at the start warm the PE HAM
    clock-gate (cold = 1.2GHz, warm = 2.4GHz) while the first DMAs
    land, so real matmuls start at full clock.
  - PSUM plan (8 banks): scores "at" 2x[128,1024] (4) + proj "pj"
    2x[128,512] (2) + acc 1x[65,1024] (2). Proj/fin/V-transpose tiles
    rotate through the dedicated "pj" pool so they never wait behind
    the exp of an old scores tile (v1 lost ~6us to that).
    Scores at_e/at_o are single-buffered per pair; the e-tile of the
    next pair only needs exp(e) of the current pair done, which the
    ACT-bound pipeline satisfies with ~1us of slack.
  - Output DMA per 4-block fin group, on the scalar HWDGE queue.

Device dataflow per core, matmuls fp32r (1 cyc/col warm) or bf16:
    scores: A^T per k-tile pair as two concurrent row-half matmuls
    (contraction dh=64), P^T = exp(A^T - 40) bf16 on ACT (no row max:
    |A| <= ~77), AV: acc[65, q] += V_aug^T @ P^T over k-tiles (col 64
    of V_aug = ones => rowsum), finalize out = PE-transpose(acc) /
    (8 * rowsum).
"""

import sys

import numpy as np

for _p in ("/opt/trn_rl_repo",):
    if _p not in sys.path:
        sys.path.insert(0, _p)

import concourse.mybir as mybir  # noqa: E402
import concourse.tile as tile  # noqa: E402
from concourse import bacc  # noqa: E402
from concourse.bass_utils import run_bass_kernel_spmd  # noqa: E402
from concourse.masks import make_identity  # noqa: E402

B, S, D, DH = 4, 4096, 768, 64
QN = S // 2          # queries per core
NSB = 8              # 512-wide super-blocks of s
NKT = 32             # 128-wide key tiles
NPAIR = NKT // 2
HALF = 1024          # q-chunk for the main loop
EXP_BIAS = -40.0     # global score offset (softmax-invariant), fp32 headroom
NWARM = 12           # HAM warm-up matmuls

F32 = mybir.dt.float32
F32R = mybir.dt.float32r
BF16 = mybir.dt.bfloat16
ADD = mybir.AluOpType.add

_NC_CACHE = None
LAST_RESULTS = None


def _build():
    nc = bacc.Bacc(num_devices=8)
    # xt: dual-row chunk layout [128, 6, S]; partition p<64 holds D-chunk
    # 2j element p, p>=64 holds chunk 2j+1 element p-64.
    xt_d = nc.dram_tensor("xt", [128, 6, S], F32R, kind="ExternalInput")
    # wt: same row layout; free cols 0:64 K, 64:128 V, 128:192 Q, 192:256 Q.
    wt_d = nc.dram_tensor("wt", [128, 6, 256], F32R, kind="ExternalInput")
    out_d = nc.dram_tensor("out", [QN, DH], F32, kind="ExternalOutput")

    with tile.TileContext(nc) as tc:
        with (
            tc.tile_pool(name="big", bufs=1) as big,
            tc.tile_pool(name="psat", bufs=2, space="PSUM") as psat,
            tc.tile_pool(name="pspj", bufs=2, space="PSUM") as pspj,
            tc.tile_pool(name="psacc", bufs=1, space="PSUM") as psacc,
            tc.tile_pool(name="pt", bufs=6) as ptp,
            tc.tile_pool(name="small", bufs=4) as small,
        ):
            xt_tiles = []
            for _sb in range(NSB):
                _xt = big.tile([128, 6, 512], F32R, tag=f"xt{_sb}")
                xt_tiles.append(_xt)
            wt_sb = big.tile([128, 6, 256], F32R)
            ktp = big.tile([128, NPAIR * 128], F32R)  # pair-interleaved K^T
            qt_sb = big.tile([128, QN], F32R)         # Q^T duplicated rows
            vt_sb = big.tile([64, S], BF16)
            v_sb = big.tile([128, NKT, 80], BF16)     # [...,0:64]=V, 64=ones
            acc_sb = big.tile([65, QN], F32)
            osb = big.tile([128, 16, DH], F32)
            ident = big.tile([128, 128], F32)
            identb = big.tile([128, 128], BF16)

            make_identity(nc, ident[:])
            make_identity(nc, identb[:])
            nc.vector.memset(v_sb[:, :, 64:65], 1.0)
            ebias = big.tile([128, 1], F32)
            nc.vector.memset(ebias[:], EXP_BIAS)

            # ---- pre-issue all input DMAs in consumption order ----
            nc.sync.dma_start(out=wt_sb[:], in_=wt_d[:])
            for kk in range(3):
                nc.sync.dma_start(
                    out=xt_tiles[0][:, 2 * kk:2 * kk + 2, :],
                    in_=xt_d[:, 2 * kk:2 * kk + 2, 0:512],
                )
            for sb in range(1, NSB):
                nc.sync.dma_start(
                    out=xt_tiles[sb][:],
                    in_=xt_d[:, :, sb * 512:(sb + 1) * 512],
                )

            # ---- HAM warm-up: dummy matmuls on the bf16 identity tile ----
            for w in range(NWARM):
                wps = pspj.tile([128, 512], F32, tag="pj")
                nc.tensor.matmul(
                    wps[:, 0:128], identb[:, 0:128], identb[:, 0:128],
                    start=True, stop=True,
                )

            def emit_proj_kv(sb):
                # Even D-chunks contract on PE rows 0:64, odd on 64:128;
                # the two psum partials are combined by DVE copy-then-add
                # (PSUM has a single DVE read port, so no psum+psum add).
                xt_t = xt_tiles[sb]
                kv_e = pspj.tile([128, 512], F32, tag="pj")
                kv_o = pspj.tile([128, 512], F32, tag="pj")
                for j in range(6):
                    nc.tensor.matmul(
                        kv_e[:, :],
                        wt_sb[0:64, j, 0:128],
                        xt_t[0:64, j, :],
                        start=(j == 0), stop=(j == 5),
                    )
                    nc.tensor.matmul(
                        kv_o[:, :],
                        wt_sb[64:128, j, 0:128],
                        xt_t[64:128, j, :],
                        start=(j == 0), stop=(j == 5),
                    )
                sl = slice(sb * 512, (sb + 1) * 512)
                kv_er = kv_e[:].bitcast(F32R)
                kv_or = kv_o[:].bitcast(F32R)
                for i in range(4):
                    t = 4 * sb + i
                    rh = 64 * (t % 2)
                    dst = ktp[rh:rh + 64, (t // 2) * 128:(t // 2 + 1) * 128]
                    nc.vector.tensor_copy(dst, kv_er[0:64, i * 128:(i + 1) * 128])
                    nc.vector.tensor_tensor(
                        out=dst, in0=dst,
                        in1=kv_or[0:64, i * 128:(i + 1) * 128], op=ADD,
                    )
                nc.vector.tensor_copy(vt_sb[:, sl], kv_e[64:128, :])
                nc.vector.tensor_tensor(
                    out=vt_sb[:, sl], in0=vt_sb[:, sl], in1=kv_o[64:128, :],
                    op=ADD,
                )

            def emit_proj_q(sb):
                xt_t = xt_tiles[sb]
                q_e = pspj.tile([128, 512], F32, tag="pj")
                q_o = pspj.tile([128, 512], F32, tag="pj")
                for j in range(6):
                    nc.tensor.matmul(
                        q_e[:, :],
                        wt_sb[0:64, j, 128:256],
                        xt_t[0:64, j, :],
                        start=(j == 0), stop=(j == 5),
                    )
                    nc.tensor.matmul(
                        q_o[:, :],
                        wt_sb[64:128, j, 128:256],
                        xt_t[64:128, j, :],
                        start=(j == 0), stop=(j == 5),
                    )
                sl = slice(sb * 512, (sb + 1) * 512)
                nc.vector.tensor_copy(qt_sb[:, sl], q_e[:].bitcast(F32R))
                nc.vector.tensor_tensor(
                    out=qt_sb[:, sl], in0=qt_sb[:, sl],
                    in1=q_o[:].bitcast(F32R), op=ADD,
                )

            def emit_vtrans(sb):
                # V natural tiles via PE transpose (bf16), into a pj slot
                tp = pspj.tile([128, 512], F32, tag="pj")
                t16 = tp[:].bitcast(BF16)
                for i in range(4):
                    t = 4 * sb + i
                    nc.tensor.transpose(
                        t16[:, i * 64:(i + 1) * 64],
                        vt_sb[:, t * 128:(t + 1) * 128],
                        identb[0:64, 0:64],
                    )
                    nc.vector.tensor_copy(
                        v_sb[:, t, 0:64], t16[:, i * 64:(i + 1) * 64],
                    )

            pts = {}
            accs = {}

            def emit_scores(h, j):
                at_e = psat.tile([128, HALF], F32, tag="at")
                at_o = psat.tile([128, HALF], F32, tag="at")
                for g in range(2):
                    gsl = slice(h * HALF + g * 512, h * HALF + g * 512 + 512)
                    osl = slice(g * 512, (g + 1) * 512)
                    nc.tensor.matmul(
                        at_e[:, osl],
                        ktp[0:64, j * 128:(j + 1) * 128],
                        qt_sb[0:64, gsl],
                        start=True, stop=True,
                    )
                    nc.tensor.matmul(
                        at_o[:, osl],
                        ktp[64:128, j * 128:(j + 1) * 128],
                        qt_sb[64:128, gsl],
                        start=True, stop=True,
                    )
                pt_e = ptp.tile([128, HALF], BF16, tag="pt")
                pt_o = ptp.tile([128, HALF], BF16, tag="pt")
                nc.scalar.activation(
                    out=pt_e[:], in_=at_e[:],
                    func=mybir.ActivationFunctionType.Exp, bias=ebias[:],
                )
                nc.scalar.activation(
                    out=pt_o[:], in_=at_o[:],
                    func=mybir.ActivationFunctionType.Exp, bias=ebias[:],
                )
                pts[(h, j)] = (pt_e, pt_o)

            def emit_av(h, j):
                pt_e, pt_o = pts.pop((h, j))
                acc = accs[h]
                for tt, pt in ((2 * j, pt_e), (2 * j + 1, pt_o)):
                    for g in range(2):
                        osl = slice(g * 512, (g + 1) * 512)
                        nc.tensor.matmul(
                            acc[:, osl],
                            v_sb[:, tt, 0:65],
                            pt[:, osl],
                            start=(j == 0 and tt == 2 * j),
                            stop=(j == NPAIR - 1 and tt == 2 * j + 1),
                            skip_group_check=True,
                        )

            def emit_fin(gblk):
                ot = pspj.tile([128, 512], F32, tag="pj")
                nc.tensor.transpose(
                    ot[:, 0:65],
                    acc_sb[:, gblk * 128:(gblk + 1) * 128],
                    ident[0:65, 0:65],
                )
                r = small.tile([128, 1], F32, tag="r")
                nc.vector.reciprocal(r[:], ot[:, 64:65])
                nc.vector.tensor_scalar(
                    osb[:, gblk, :], ot[:, 0:64], r[:], 0.125,
                    op0=mybir.AluOpType.mult, op1=mybir.AluOpType.mult,
                )

            def emit_out_dma(oc):
                nc.sync.dma_start(
                    out=out_d[:].rearrange("(t p) d -> p t d", p=128)[
                        :, 4 * oc:4 * (oc + 1), :],
                    in_=osb[:, 4 * oc:4 * (oc + 1), :],
                )

            # ---- emission schedule ----
            # ramp: sb0 KV+Q, sb1 Q (scores(0,0) needs q cols 0:1024),
            # then sb1 KV + V-transposes of sb0 inside pair 0.
            emit_proj_kv(0)
            emit_proj_q(0)
            emit_proj_q(1)
            emit_vtrans(0)

            # per-pair proj units: at pair index i emit unit proj_at[i]
            proj_at = {}
            for s in range(2, NSB):
                proj_at[2 * (s - 1)] = ("kv", s)
                proj_at[2 * (s - 1) + 1] = ("vt", s)
            proj_at[14] = ("q", 2)
            proj_at[15] = ("q", 3)

            pairs = [(0, j) for j in range(NPAIR)] + [(1, j) for j in range(NPAIR)]
            acc_h0 = psacc.tile([65, HALF], F32, tag="acc")
            accs[0] = acc_h0

            for i, (h, j) in enumerate(pairs):
                unit = proj_at.get(i)
                if unit is not None:
                    kind, s = unit
                    if kind == "kv":
                        emit_proj_kv(s)
                    elif kind == "vt":
                        emit_vtrans(s)
                    else:
                        emit_proj_q(s)
                emit_scores(h, j)
                if i == 0:
                    emit_proj_kv(1)
                    emit_vtrans(1)
                if i > 0:
                    ph, pj = pairs[i - 1]
                    emit_av(ph, pj)
                    if (ph, pj) == (0, NPAIR - 1):
                        # h0 accumulation complete: stage it and swap acc
                        nc.vector.tensor_copy(acc_sb[:, 0:512], accs[0][:, 0:512])
                        nc.vector.tensor_copy(acc_sb[:, 512:HALF], accs[0][:, 512:HALF])
                        acc_h1 = psacc.tile([65, HALF], F32, tag="acc")
                        accs[1] = acc_h1
                # h0 finalize (blocks 0..7) interleaved into h1 pair stream
                if h == 1 and j in (3, 5, 7, 9):
                    for blk in range(j - 3, j - 1):
                        emit_fin(blk)
                if h == 1 and j == 10:
                    emit_out_dma(0)
                if h == 1 and j == 12:
                    emit_out_dma(1)
            emit_av(*pairs[-1])
            nc.vector.tensor_copy(acc_sb[:, HALF:HALF + 512], accs[1][:, 0:512])
            for gblk in range(8, 12):
                emit_fin(gblk)
            emit_out_dma(2)
            nc.vector.tensor_copy(acc_sb[:, HALF + 512:2 * HALF],
                                  accs[1][:, 512:HALF])
            for gblk in range(12, 16):
                emit_fin(gblk)
            emit_out_dma(3)

    nc.finalize()
    return nc


def _get_nc():
    global _NC_CACHE
    if _NC_CACHE is None:
        _NC_CACHE = _build()
    return _NC_CACHE


def kernel(x, W, _trace=False):
    global LAST_RESULTS
    x = np.ascontiguousarray(np.asarray(x), dtype=np.float32)
    W = np.ascontiguousarray(np.asarray(W), dtype=np.float32)
    assert x.shape == (B, S, D) and W.shape == (3 * DH, D)

    # wt free cols: [K | V | Q | Qdup]; dual-row chunk layout:
    # host [d, e] -> chunks of 64 along d; chunk 2j -> partitions 0:64
    # at free j, chunk 2j+1 -> partitions 64:128.
    wtf = np.concatenate(
        [W[DH:2 * DH], W[2 * DH:], W[:DH], W[:DH]], axis=0
    ).T  # [768, 256]
    wt = np.ascontiguousarray(
        wtf.reshape(6, 2, 64, 256).transpose(1, 2, 0, 3).reshape(128, 6, 256)
    )

    in_maps = []
    for c in range(8):
        b, qh = divmod(c, 2)
        xtb = x[b].T  # [768, 4096]
        if qh:
            xtb = np.concatenate([xtb[:, QN:], xtb[:, :QN]], axis=1)
        xtc = np.ascontiguousarray(
            xtb.reshape(6, 2, 64, S).transpose(1, 2, 0, 3).reshape(128, 6, S)
        )
        in_maps.append({"xt": xtc, "wt": wt})

    nc = _get_nc()
    res = run_bass_kernel_spmd(nc, in_maps, list(range(8)), trace=_trace)
    LAST_RESULTS = res

    out = np.empty((B, S, DH), np.float32)
    for c in range(8):
        b, qh = divmod(c, 2)
        out[b, qh * QN:(qh + 1) * QN] = res.results[c]["out"]
    return out


# revision 11
# speedup vs baseline: 1.1396x; 1.1396x over previous
"""Fused single-head attention (QKV proj + softmax*scale + AV) on 8 trn2 cores.

Reference computation (fp32):
    qkv = x @ W.T            x:[4,4096,768]  W:[192,768]
    q,k,v = split(qkv, 64)
    A = q @ k.T              (no pre-softmax scale)
    out = softmax(A) / 8 @ v

Sharding: core c handles batch b=c//2, query half qh=c%2 (2048 queries),
full 4096 keys of that batch. SPMD-uniform program: the host rolls the
key/value columns of x^T by qh*2048 so every core's own queries are
always columns 0:2048 (softmax is permutation-invariant over keys).

v2 design notes (vs v1 baseline at 125us):
  - Dual-row projection: D=768 split into 12 chunks of 64; even chunks
    live on SBUF partitions 0:64, odd on 64:128. The per-chunk proj
    matmuls contract only 64 partitions, so even/odd chunk matmuls run
    CONCURRENTLY in the two PE row-halves (tile_position row groups),
    halving projection wall time. The pair of PSUM partials is summed
    by the DVE add that replaces v1's PSUM->SBUF copy (same DVE cost).
  - Q duplication (scores rhs needs Q^T on both row halves) is done by
    duplicating the Q columns of W (M=128 lhsT) instead of a second
    DVE copy.
  - All input DMAs pre-issued at program start on the sync HWDGE queue
    in consumption order (wt, sb0 in 3 chunks, sb1..sb7 whole), so the
    queue streams back-to-back instead of being paced by emission.
  - ~3.5us of dummy identity matmuls at the start warm the PE HAM
    clock-gate (cold = 1.2GHz, warm = 2.4GHz) while the first DMAs
    land, so real matmuls start at full clock.
  - PSUM plan (8 banks): scores "at" 2x[128,1024] (4) + proj "pj"
    2x[128,512] (2) + acc 1x[65,1024] (2). Proj/fin/V-transpose tiles
    rotate through the dedicated "pj" pool so they never wait behind
    the exp of an old scores tile (v1 lost ~6us to that).
    Scores at_e/at_o are single-buffered per pair; the e-tile of the
    next pair only needs exp(e) of the current pair done, which the
    ACT-bound pipeline satisfies with ~1us of slack.
  - Output DMA per 4-block fin group, on the scalar HWDGE queue.

Device dataflow per core, matmuls fp32r (1 cyc/col warm) or bf16:
    scores: A^T per k-tile pair as two concurrent row-half matmuls
    (contraction dh=64), P^T = exp(A^T - 40) bf16 on ACT (no row max:
    |A| <= ~77), AV: acc[65, q] += V_aug^T @ P^T over k-tiles (col 64
    of V_aug = ones => rowsum), finalize out = PE-transpose(acc) /
    (8 * rowsum).
"""

import sys

import numpy as np

for _p in ("/opt/trn_rl_repo",):
    if _p not in sys.path:
        sys.path.insert(0, _p)

import concourse.mybir as mybir  # noqa: E402
import concourse.tile as tile  # noqa: E402
from concourse import bacc  # noqa: E402
from concourse.bass_utils import run_bass_kernel_spmd  # noqa: E402
from concourse.masks import make_identity  # noqa: E402

B, S, D, DH = 4, 4096, 768, 64
QN = S // 2          # queries per core
NSB = 8              # 512-wide super-blocks of s
NKT = 32             # 128-wide key tiles
NPAIR = NKT // 2
HALF = 1024          # q-chunk for the main loop
EXP_BIAS = -40.0     # global score offset (softmax-invariant), fp32 headroom
NWARM = 12           # HAM warm-up matmuls

F32 = mybir.dt.float32
F32R = mybir.dt.float32r
BF16 = mybir.dt.bfloat16
ADD = mybir.AluOpType.add

_NC_CACHE = None
LAST_RESULTS = None


def _build():
    nc = bacc.Bacc(num_devices=8)
    # xt: dual-row chunk layout [128, 6, S]; partition p<64 holds D-chunk
    # 2j element p, p>=64 holds chunk 2j+1 element p-64.
    xt_d = nc.dram_tensor("xt", [128, 6, S], F32R, kind="ExternalInput")
    # wt: same row layout; free cols 0:64 K, 64:128 V, 128:192 Q, 192:256 Q.
    wt_d = nc.dram_tensor("wt", [128, 6, 256], F32R, kind="ExternalInput")
    out_d = nc.dram_tensor("out", [QN, DH], F32, kind="ExternalOutput")

    with tile.TileContext(nc) as tc:
        with (
            tc.tile_pool(name="big", bufs=1) as big,
            tc.tile_pool(name="psat", bufs=2, space="PSUM") as psat,
            tc.tile_pool(name="pspj", bufs=2, space="PSUM") as pspj,
            tc.tile_pool(name="psacc", bufs=1, space="PSUM") as psacc,
            tc.tile_pool(name="pt", bufs=6) as ptp,
            tc.tile_pool(name="small", bufs=4) as small,
        ):
            xt_tiles = []
            for _sb in range(NSB):
                _xt = big.tile([128, 6, 512], F32R, tag=f"xt{_sb}")
                xt_tiles.append(_xt)
            wt_sb = big.tile([128, 6, 256], F32R)
            ktp = big.tile([128, NPAIR * 128], F32R)  # pair-interleaved K^T
            qt_sb = big.tile([128, QN], F32R)         # Q^T duplicated rows
            vt_sb = big.tile([64, S], BF16)
            v_sb = big.tile([128, NKT, 80], BF16)     # [...,0:64]=V, 64=ones
            acc_sb = big.tile([65, QN], F32)
            osb = big.tile([128, 16, DH], F32)
            ident = big.tile([128, 128], F32)
            identb = big.tile([128, 128], BF16)

            make_identity(nc, ident[:])
            make_identity(nc, identb[:])
            nc.vector.memset(v_sb[:, :, 64:65], 1.0)
            ebias = big.tile([128, 1], F32)
            nc.vector.memset(ebias[:], EXP_BIAS)

            # ---- pre-issue all input DMAs in consumption order ----
            nc.sync.dma_start(out=wt_sb[:], in_=wt_d[:])
            for kk in range(3):
                nc.sync.dma_start(
                    out=xt_tiles[0][:, 2 * kk:2 * kk + 2, :],
                    in_=xt_d[:, 2 * kk:2 * kk + 2, 0:512],
                )
            for sb in range(1, NSB):
                nc.sync.dma_start(
                    out=xt_tiles[sb][:],
                    in_=xt_d[:, :, sb * 512:(sb + 1) * 512],
                )

            # ---- HAM warm-up: ~4us of sustained dummy matmuls so the PE
            # clock-gate opens (cold=1.2GHz) before real work arrives.
            wsrc = big.tile([128, 512], BF16)
            nc.vector.memset(wsrc[:], 0.0)
            for w in range(NWARM):
                wps = pspj.tile([128, 512], F32, tag="pj")
                nc.tensor.matmul(
                    wps[:, :], identb[:, 0:128], wsrc[:],
                    start=True, stop=True,
                )

            def emit_proj_kv(sb):
                # Full 128-row contraction; free index j covers D chunk-pair
                # (2j, 2j+1) of the permuted layout.
                xt_t = xt_tiles[sb]
                kv_ps = pspj.tile([128, 512], F32, tag="pj")
                for j in range(6):
                    nc.tensor.matmul(
                        kv_ps[:, :],
                        wt_sb[:, j, 0:128],
                        xt_t[:, j, :],
                        start=(j == 0), stop=(j == 5),
                    )
                sl = slice(sb * 512, (sb + 1) * 512)
                for i in range(4):
                    t = 4 * sb + i
                    rh = 64 * (t % 2)
                    nc.vector.tensor_copy(
                        ktp[rh:rh + 64, (t // 2) * 128:(t // 2 + 1) * 128],
                        kv_ps[0:64, i * 128:(i + 1) * 128],
                    )
                nc.vector.tensor_copy(vt_sb[:, sl], kv_ps[64:128, :])

            def emit_proj_q(sb):
                xt_t = xt_tiles[sb]
                q_ps = pspj.tile([128, 512], F32, tag="pj")
                for j in range(6):
                    nc.tensor.matmul(
                        q_ps[:, :],
                        wt_sb[:, j, 128:256],
                        xt_t[:, j, :],
                        start=(j == 0), stop=(j == 5),
                    )
                sl = slice(sb * 512, (sb + 1) * 512)
                nc.vector.tensor_copy(qt_sb[:, sl], q_ps[:])

            def emit_vtrans(sb):
                # V natural tiles via PE transpose (bf16), into a pj slot
                tp = pspj.tile([128, 512], F32, tag="pj")
                t16 = tp[:].bitcast(BF16)
                for i in range(4):
                    t = 4 * sb + i
                    nc.tensor.transpose(
                        t16[:, i * 64:(i + 1) * 64],
                        vt_sb[:, t * 128:(t + 1) * 128],
                        identb[0:64, 0:64],
                    )
                    nc.vector.tensor_copy(
                        v_sb[:, t, 0:64], t16[:, i * 64:(i + 1) * 64],
                    )

            pts = {}
            accs = {}

            def emit_scores(h, j):
                at_e = psat.tile([128, HALF], F32, tag="at")
                at_o = psat.tile([128, HALF], F32, tag="at")
                for g in range(2):
                    gsl = slice(h * HALF + g * 512, h * HALF + g * 512 + 512)
                    osl = slice(g * 512, (g + 1) * 512)
                    nc.tensor.matmul(
                        at_e[:, osl],
                        ktp[0:64, j * 128:(j + 1) * 128],
                        qt_sb[0:64, gsl],
                        start=True, stop=True,
                    )
                    nc.tensor.matmul(
                        at_o[:, osl],
                        ktp[64:128, j * 128:(j + 1) * 128],
                        qt_sb[64:128, gsl],
                        start=True, stop=True,
                    )
                pt_e = ptp.tile([128, HALF], BF16, tag="pt")
                pt_o = ptp.tile([128, HALF], BF16, tag="pt")
                nc.scalar.activation(
                    out=pt_e[:], in_=at_e[:],
                    func=mybir.ActivationFunctionType.Exp, bias=ebias[:],
                )
                nc.scalar.activation(
                    out=pt_o[:], in_=at_o[:],
                    func=mybir.ActivationFunctionType.Exp, bias=ebias[:],
                )
                pts[(h, j)] = (pt_e, pt_o)

            def emit_av(h, j):
                pt_e, pt_o = pts.pop((h, j))
                acc = accs[h]
                for tt, pt in ((2 * j, pt_e), (2 * j + 1, pt_o)):
                    for g in range(2):
                        osl = slice(g * 512, (g + 1) * 512)
                        nc.tensor.matmul(
                            acc[:, osl],
                            v_sb[:, tt, 0:65],
                            pt[:, osl],
                            start=(j == 0 and tt == 2 * j),
                            stop=(j == NPAIR - 1 and tt == 2 * j + 1),
                            skip_group_check=True,
                        )

            def emit_fin(gblk):
                ot = pspj.tile([128, 512], F32, tag="pj")
                nc.tensor.transpose(
                    ot[:, 0:65],
                    acc_sb[:, gblk * 128:(gblk + 1) * 128],
                    ident[0:65, 0:65],
                )
                r = small.tile([128, 1], F32, tag="r")
                nc.vector.reciprocal(r[:], ot[:, 64:65])
                nc.vector.tensor_scalar(
                    osb[:, gblk, :], ot[:, 0:64], r[:], 0.125,
                    op0=mybir.AluOpType.mult, op1=mybir.AluOpType.mult,
                )

            def emit_out_dma(oc):
                nc.sync.dma_start(
                    out=out_d[:].rearrange("(t p) d -> p t d", p=128)[
                        :, 4 * oc:4 * (oc + 1), :],
                    in_=osb[:, 4 * oc:4 * (oc + 1), :],
                )

            # ---- emission schedule ----
            # ramp: sb0 KV+Q, sb1 Q (scores(0,0) needs q cols 0:1024),
            # then sb1 KV + V-transposes of sb0 inside pair 0.
            emit_proj_kv(0)
            emit_proj_q(0)
            emit_proj_q(1)
            emit_vtrans(0)

            # per-pair proj units: at pair index i emit unit proj_at[i]
            proj_at = {}
            for s in range(2, NSB):
                proj_at[2 * (s - 1)] = ("kv", s)
                proj_at[2 * (s - 1) + 1] = ("vt", s)
            proj_at[14] = ("q", 2)
            proj_at[15] = ("q", 3)

            pairs = [(0, j) for j in range(NPAIR)] + [(1, j) for j in range(NPAIR)]
            acc_h0 = psacc.tile([65, HALF], F32, tag="acc")
            accs[0] = acc_h0

            for i, (h, j) in enumerate(pairs):
                unit = proj_at.get(i)
                if unit is not None:
                    kind, s = unit
                    if kind == "kv":
                        emit_proj_kv(s)
                    elif kind == "vt":
                        emit_vtrans(s)
                    else:
                        emit_proj_q(s)
                emit_scores(h, j)
                if i == 0:
                    emit_proj_kv(1)
                    emit_vtrans(1)
                if i > 0:
                    ph, pj = pairs[i - 1]
                    emit_av(ph, pj)
                    if (ph, pj) == (0, NPAIR - 1):
                        # h0 accumulation complete: stage it and swap acc
                        nc.vector.tensor_copy(acc_sb[:, 0:512], accs[0][:, 0:512])
                        nc.vector.tensor_copy(acc_sb[:, 512:HALF], accs[0][:, 512:HALF])
                        acc_h1 = psacc.tile([65, HALF], F32, tag="acc")
                        accs[1] = acc_h1
                # h0 finalize (blocks 0..7) interleaved into h1 pair stream
                if h == 1 and j in (3, 5, 7, 9):
                    for blk in range(j - 3, j - 1):
                        emit_fin(blk)
                if h == 1 and j == 10:
                    emit_out_dma(0)
                if h == 1 and j == 12:
                    emit_out_dma(1)
            emit_av(*pairs[-1])
            nc.vector.tensor_copy(acc_sb[:, HALF:HALF + 512], accs[1][:, 0:512])
            for gblk in range(8, 12):
                emit_fin(gblk)
            emit_out_dma(2)
            nc.vector.tensor_copy(acc_sb[:, HALF + 512:2 * HALF],
                                  accs[1][:, 512:HALF])
            for gblk in range(12, 16):
                emit_fin(gblk)
            emit_out_dma(3)

    nc.finalize()
    return nc


def _get_nc():
    global _NC_CACHE
    if _NC_CACHE is None:
        _NC_CACHE = _build()
    return _NC_CACHE


def kernel(x, W, _trace=False):
    global LAST_RESULTS
    x = np.ascontiguousarray(np.asarray(x), dtype=np.float32)
    W = np.ascontiguousarray(np.asarray(W), dtype=np.float32)
    assert x.shape == (B, S, D) and W.shape == (3 * DH, D)

    # wt free cols: [K | V | Q | Qdup]; dual-row chunk layout:
    # host [d, e] -> chunks of 64 along d; chunk 2j -> partitions 0:64
    # at free j, chunk 2j+1 -> partitions 64:128.
    wtf = np.concatenate(
        [W[DH:2 * DH], W[2 * DH:], W[:DH], W[:DH]], axis=0
    ).T  # [768, 256]
    wt = np.ascontiguousarray(
        wtf.reshape(6, 2, 64, 256).transpose(1, 2, 0, 3).reshape(128, 6, 256)
    )

    in_maps = []
    for c in range(8):
        b, qh = divmod(c, 2)
        xtb = x[b].T  # [768, 4096]
        if qh:
            xtb = np.concatenate([xtb[:, QN:], xtb[:, :QN]], axis=1)
        xtc = np.ascontiguousarray(
            xtb.reshape(6, 2, 64, S).transpose(1, 2, 0, 3).reshape(128, 6, S)
        )
        in_maps.append({"xt": xtc, "wt": wt})

    nc = _get_nc()
    res = run_bass_kernel_spmd(nc, in_maps, list(range(8)), trace=_trace)
    LAST_RESULTS = res

    out = np.empty((B, S, DH), np.float32)
    for c in range(8):
        b, qh = divmod(c, 2)
        out[b, qh * QN:(qh + 1) * QN] = res.results[c]["out"]
    return out
